# revision 56
# baseline (speedup 1.0000x reference)
"""Trainium2 Bass kernel for nn_Attention_53463752901338.

Computes K = rope(x @ Wk.T + bk), Q = rope(x @ Wq.T + bq), V = x @ Wv.T + bv
with x (16, 1024, 2048), W* (2048, 2048), b* (2048,).

Strategy: data-parallel over batch — each of the 8 NeuronCores gets 2 of the
16 batches (2048 tokens) and all three weight matrices; no collectives.
Matmuls run in bf16 (fp32 accumulate in PSUM); RoPE + bias run in fp32 on the
vector/scalar engines fused into the PSUM->SBUF evacuation.  The kernel is
PE-stream-bound: 3072 N=512 matmuls at 216 ns ≈ 663 us; everything else
(DMA supply, clock warm-up, epilogues, output drain) is hidden behind it.

Host-side prep (untimed): x is transposed to (d, tok) per core and cast bf16;
W for K/Q is row-permuted so the even/odd feature deinterleave of RoPE becomes
two contiguous halves; weights are laid out per (chunk-pair, k-group); biases
are bf16, broadcast to 128 rows, and grouped [even|odd] per pair so each
epilogue bias add is a single contiguous 1024-wide op.
"""

import sys

if "/opt/trn_rl_repo" not in sys.path:
    sys.path.insert(0, "/opt/trn_rl_repo")

import numpy as np
import ml_dtypes

import concourse.bass as bass
import concourse.mybir as mybir
import concourse.tile as tile
from concourse import bacc
from concourse.bass_utils import run_bass_kernel_spmd

B, S, D = 16, 1024, 2048
N_CORES = 8
TOK = B * S // N_CORES          # 2048 tokens per core
KT = D // 128                   # 16 contraction tiles
NT = TOK // 128                 # 16 token tiles per core
BF16 = mybir.dt.bfloat16
F32 = mybir.dt.float32
NPBF16 = ml_dtypes.bfloat16

_COMPILED = None


def _build():
    nc = bacc.Bacc("TRN2", target_bir_lowering=False, debug=False,
                   num_devices=N_CORES)

    xT_d = nc.dram_tensor("xT", (D, TOK), BF16, kind="ExternalInput")
    w_d = {p: nc.dram_tensor(f"W{p}", (2, 4, 128, 4096), BF16,
                             kind="ExternalInput") for p in "KQV"}
    b_d = {p: nc.dram_tensor(f"b{p}", (128, D), BF16, kind="ExternalInput")
           for p in "KQV"}
    cos_d = nc.dram_tensor("cos", (128, 8), F32, kind="ExternalInput")
    sin_d = nc.dram_tensor("sin", (128, 8), F32, kind="ExternalInput")
    # outputs viewed as (tok, half, 1024): half 0 = cols 0:1024, half 1 = 1024:2048
    o_d = {p: nc.dram_tensor(f"O{p}", (TOK, 2, 1024), F32,
                             kind="ExternalOutput") for p in "KQV"}

    MULT = mybir.AluOpType.mult
    ADD = mybir.AluOpType.add
    SUB = mybir.AluOpType.subtract

    with tile.TileContext(nc) as tc:
        with (
            tc.tile_pool(name="sp", bufs=1) as sp,
            tc.tile_pool(name="pp", bufs=4, space=bass.MemorySpace.PSUM) as pp,
        ):
            # All 16 DMA engines round-robin one HW queue at ~430 GB/s, so
            # a transfer's completion is stretched by everything issued
            # after it.  Keep every transfer small (x tiles split into
            # token-halves, W in per-k 256KB slices) and issue strictly in
            # consumption order: the first matmul then needs only 512KB and
            # the startup block's demand (~300 GB/s) stays below supply.
            # HAM warm-up: small (N=64) dummy matmuls on a memset tile keep
            # the PE busy from ~7.6us until the first real data lands, so
            # the clock gate is open (2.4 GHz) when real work arrives.
            warm = sp.tile([128, 192], BF16, tag="warm", name="warm")
            nc.vector.memset(warm[:], 0.0)
            ps_warm = pp.tile([128, 64], F32, tag="ps", name="ps_warm")

            def warm_mms(n):
                for _ in range(n):
                    nc.tensor.matmul(ps_warm[:], warm[:, 0:128],
                                     warm[:, 128:192], start=True, stop=True)

            warm_mms(125)

            bias_sb = {}
            xa = []          # x k-tiles, token half t=0..7
            xb = []          # x k-tiles, token half t=8..15
            w0 = []
            xq0 = []         # k=0 in two quarter tiles: t=0..3, t=4..7

            def xslice(k, t):
                if k == 0 and t < 8:
                    return xq0[t // 4][:, (t % 4) * 128:(t % 4 + 1) * 128]
                xt_ = xa[k] if t < 8 else xb[k]
                return xt_[:, (t % 8) * 128:(t % 8 + 1) * 128]

            # The startup-phase W + biasK issue from the Activation engine's
            # HW queue, x from Sync's.  Total HBM bandwidth is shared
            # (~430 GB/s) but the per-queue ~0.65us/issue serialization is
            # not: the x and W streams advance independently, so the first
            # matmul's data (xa0 + W_k0) is in flight within ~2 issues.
            # Later phases' W goes back on the Sync queue: on the ACT queue
            # those issues end up behind the previous phase's epilogue muls
            # in the in-order ACT stream and stall the phase transition.
            def w_slice_dma(proj, pair, k, eng=None):
                w_ = sp.tile([128, 1024], BF16, tag="w", bufs=32)
                g, kk = divmod(k, 4)
                (eng or nc.sync).dma_start(
                    w_[:], w_d[proj].ap()[pair, g, :,
                                          kk * 1024:(kk + 1) * 1024])
                return w_

            # 4KB primers absorb the Activation queue's first-use spin-up
            # so W_k0 behind them streams at full rate immediately
            cos_sb = sp.tile([128, 8], F32, tag="cos", name="cos_sb")
            nc.scalar.dma_start(cos_sb[:], cos_d.ap()[:])
            sin_sb = sp.tile([128, 8], F32, tag="sin", name="sin_sb")
            nc.scalar.dma_start(sin_sb[:], sin_d.ap()[:])
            for k in range(KT):
                if k == 0:
                    # first contraction tile in two 256KB quarters so the
                    # very first matmul's LDW waits on minimal data
                    for q in range(2):
                        t_ = sp.tile([128, 512], BF16, tag=f"xq{q}",
                                     name=f"xq{q}")
                        nc.sync.dma_start(
                            t_[:], xT_d.ap()[0:128, q * 512:(q + 1) * 512])
                        xq0.append(t_)
                    xa.append(None)
                else:
                    t_ = sp.tile([128, 1024], BF16, tag=f"xa{k}",
                                 name=f"xa{k}")
                    nc.sync.dma_start(
                        t_[:], xT_d.ap()[k * 128:(k + 1) * 128, 0:1024])
                    xa.append(t_)
                w0.append(w_slice_dma("K", 0, k, eng=nc.scalar))
                if k == 7:
                    # needed by the startup partial-sum drains (~24us)
                    bias_sb["K"] = sp.tile([128, D], BF16, tag="bK",
                                           name="biasK")
                    nc.scalar.dma_start(bias_sb["K"][:], b_d["K"].ap()[:])
            for k in range(KT):
                t_ = sp.tile([128, 1024], BF16, tag=f"xb{k}", name=f"xb{k}")
                nc.sync.dma_start(t_[:],
                                  xT_d.ap()[k * 128:(k + 1) * 128, 1024:2048])
                xb.append(t_)

            for proj, pair in [("K", 0), ("K", 1), ("Q", 0), ("Q", 1),
                               ("V", 0), ("V", 1)]:
                if proj not in bias_sb:
                    bias_sb[proj] = sp.tile([128, D], BF16, tag=f"b{proj}",
                                            name=f"bias{proj}")
                    nc.sync.dma_start(bias_sb[proj][:], b_d[proj].ap()[:])
                if proj == "K" and pair == 0:
                    wt = w0
                else:
                    wt = [w_slice_dma(proj, pair, k) for k in range(KT)]

                # bias is pre-layed-out per pair as [even 512 | odd 512], so
                # bia is one contiguous [128, 1024] operand: epilogues do a
                # single 1024-wide add instead of two 512-wide ones
                bia = bias_sb[proj][:, pair * 1024:(pair + 1) * 1024]
                be = bias_sb[proj][:, pair * 1024:pair * 1024 + 512]
                bo = bias_sb[proj][:, pair * 1024 + 512:(pair + 1) * 1024]

                def mm_group(ps, t, wt=wt):
                    for k in range(KT):
                        lhsT = xslice(k, t)
                        nc.tensor.matmul(
                            ps[:, 0:512], lhsT, wt[k][:, 0:512],
                            start=(k == 0), stop=(k == KT - 1))
                        nc.tensor.matmul(
                            ps[:, 512:1024], lhsT, wt[k][:, 512:1024],
                            start=(k == 0), stop=(k == KT - 1))

                def epilogue(ps, t, proj=proj, pair=pair, bia=bia):
                    oq = nc.sync
                    out_t = sp.tile([128, 2, 512], F32, tag="out",
                                    name="out_t", bufs=3)
                    if proj == "V":
                        nc.vector.tensor_add(out_t[:], ps[:, 0:1024], bia)
                    else:
                        st = t % 8
                        cos_ap = cos_sb[:, st:st + 1]
                        sin_ap = sin_sb[:, st:st + 1]
                        yb = sp.tile([128, 1024], F32, tag="yb", name="yb",
                                     bufs=3)
                        u = sp.tile([128, 512], F32, tag="u", name="u", bufs=3)
                        v = sp.tile([128, 512], F32, tag="u", name="v", bufs=3)
                        nc.vector.tensor_add(yb[:], ps[:, 0:1024], bia)
                        yeb = yb[:, 0:512]
                        yob = yb[:, 512:1024]
                        nc.scalar.mul(u[:], yob, sin_ap)
                        nc.vector.scalar_tensor_tensor(
                            out_t[:, 0, :], yeb, cos_ap, u[:], MULT, SUB)
                        nc.scalar.mul(v[:], yob, cos_ap)
                        nc.vector.scalar_tensor_tensor(
                            out_t[:, 1, :], yeb, sin_ap, v[:], MULT, ADD)

                    oq.dma_start(
                        o_d[proj].ap()[t * 128:(t + 1) * 128, :,
                                       pair * 512:(pair + 1) * 512],
                        out_t[:])

                if proj == "K" and pair == 0:
                    # Startup: while the initial 12.4 MB x/W load streams in,
                    # the in-order PE stream must have work matched to DMA
                    # arrival order.  Process t=0..7 in two half-contraction
                    # passes: k-major blocks of 4 token tiles over k=0..7
                    # (only the first 6 MB of data), partial sums parked in
                    # SBUF as bf16 (bias folded in), then the k=8..15 halves
                    # merge via the epilogue's bias operand slot.
                    sv = {}
                    for blk in range(2):
                        psA = [pp.tile([128, 1024], F32, tag="ps",
                                       name=f"psA{blk}_{i}") for i in range(4)]
                        for k in range(8):
                            for i, psi in enumerate(psA):
                                t = blk * 4 + i
                                lhsT = xslice(k, t)
                                nc.tensor.matmul(
                                    psi[:, 0:512], lhsT, wt[k][:, 0:512],
                                    start=(k == 0), stop=(k == 7))
                                nc.tensor.matmul(
                                    psi[:, 512:1024], lhsT,
                                    wt[k][:, 512:1024],
                                    start=(k == 0), stop=(k == 7))
                        for i, psi in enumerate(psA):
                            t = blk * 4 + i
                            s_ = sp.tile([128, 1024], BF16, tag=f"sv{t}",
                                          name=f"sv{t}")
                            nc.vector.tensor_add(s_[:], psi[:, 0:1024], bia)
                            sv[t] = s_
                    # Interleave each merge tile (16 MMs) with a full t>=8
                    # tile (32 MMs): a merge tile alone gives the PE only
                    # ~3.5us per ~2.8us of DVE epilogue work, which leaves
                    # no PSUM-recycle slack and can stall the PE.
                    for j in range(8):
                        ps = pp.tile([128, 1024], F32, tag="ps", name="psB")
                        for k in range(8, KT):
                            lhsT = xslice(k, j)
                            nc.tensor.matmul(
                                ps[:, 0:512], lhsT, wt[k][:, 0:512],
                                start=(k == 8), stop=(k == KT - 1))
                            nc.tensor.matmul(
                                ps[:, 512:1024], lhsT, wt[k][:, 512:1024],
                                start=(k == 8), stop=(k == KT - 1))
                        epilogue(ps, j, bia=sv[j][:])
                        ps2 = pp.tile([128, 1024], F32, tag="ps", name="ps")
                        mm_group(ps2, 8 + j)
                        epilogue(ps2, 8 + j)
                else:
                    for t in range(NT):
                        if proj == "V" and pair == 1 and t == NT - 1:
                            # final tile: half-column passes in separate
                            # PSUM tiles so the first half's epilogue + DMA
                            # overlap the second half's matmuls
                            out_f = sp.tile([128, 2, 512], F32, tag="out",
                                            name="out_f", bufs=3)
                            for h, bias_h in ((0, be), (1, bo)):
                                psh = pp.tile([128, 512], F32, tag="ps",
                                              name=f"psf{h}")
                                for k in range(KT):
                                    nc.tensor.matmul(
                                        psh[:], xslice(k, t),
                                        wt[k][:, h * 512:(h + 1) * 512],
                                        start=(k == 0), stop=(k == KT - 1))
                                if h == 0:
                                    nc.vector.tensor_add(
                                        out_f[:, 0, :], psh[:], bias_h)
                                    nc.sync.dma_start(
                                        o_d[proj].ap()[
                                            t * 128:(t + 1) * 128, 0:1,
                                            pair * 512:(pair + 1) * 512],
                                        out_f[:, 0:1, :])
                                    continue
                                # very last half: quarter epilogues, DMAs on
                                # both queues, to shorten the post-matmul
                                # critical path of the whole kernel
                                base = pair * 512
                                for q, oqf in ((0, nc.sync), (1, nc.scalar)):
                                    c0, c1 = q * 256, (q + 1) * 256
                                    bq = bias_sb[proj][
                                        :, pair * 1024 + 512 + c0:
                                        pair * 1024 + 512 + c1]
                                    nc.vector.tensor_add(
                                        out_f[:, 1, c0:c1],
                                        psh[:, c0:c1], bq)
                                    oqf.dma_start(
                                        o_d[proj].ap()[
                                            t * 128:(t + 1) * 128, 1:2,
                                            base + c0:base + c1],
                                        out_f[:, 1:2, c0:c1])
                            continue
                        ps = pp.tile([128, 1024], F32, tag="ps", name="ps")
                        mm_group(ps, t)
                        epilogue(ps, t)

    nc.compile()
    return nc


def _get_compiled():
    global _COMPILED
    if _COMPILED is None:
        _COMPILED = _build()
    return _COMPILED


def _prep_weight(W, rope_perm):
    """(D, D) f32 nn.Linear weight -> (2, 4, 128, 4096) bf16 device layout.

    Output feature chunks c = fo//512; pair 0 holds chunks (0, 2), pair 1
    holds (1, 3), each k-group g holds k-tiles 4g..4g+3 laid out
    [partition][kk][512 e-cols, 512 o-cols... ] as [128, kk*1024 + c_half*512].
    """
    Wp = np.concatenate([W[0::2, :], W[1::2, :]], axis=0) if rope_perm else W
    WT = np.ascontiguousarray(Wp.T)                      # (d_in, fo)
    WTr = WT.reshape(KT, 128, 4, 512)                    # (k, row, chunk, col)
    pairs = np.stack([WTr[:, :, [0, 2], :], WTr[:, :, [1, 3], :]], axis=0)
    dev = pairs.reshape(2, KT, 128, 1024)                # (pair, k, row, 1024)
    dev = dev.reshape(2, 4, 4, 128, 1024).transpose(0, 1, 3, 2, 4)
    dev = np.ascontiguousarray(dev.reshape(2, 4, 128, 4096))
    return dev.astype(NPBF16)


def _prep_bias(b, rope_perm):
    bp = np.concatenate([b[0::2], b[1::2]]) if rope_perm else b
    # group per pair as [even 512 | odd 512] so each pair's bias is one
    # contiguous [128, 1024] operand on device
    bp = np.concatenate([bp[0:512], bp[1024:1536], bp[512:1024],
                         bp[1536:2048]])
    return np.ascontiguousarray(
        np.broadcast_to(bp.astype(NPBF16), (128, D)))


def _prep_inputs(x, Wk, bk, Wq, bq, Wv, bv):
    inv_freq = 1.0 / (10000.0 ** (
        np.arange(0.0, D, 2.0, dtype=np.float32) / np.float32(D)))
    freqs = inv_freq * np.arange(S, dtype=np.float32)
    cos = np.cos(freqs).astype(np.float32)               # (1024,)
    sin = np.sin(freqs).astype(np.float32)
    cos_t = np.ascontiguousarray(cos.reshape(8, 128).T)  # (128, 8)
    sin_t = np.ascontiguousarray(sin.reshape(8, 128).T)

    shared = {
        "WK": _prep_weight(Wk, True),
        "WQ": _prep_weight(Wq, True),
        "WV": _prep_weight(Wv, False),
        "bK": _prep_bias(bk, True),
        "bQ": _prep_bias(bq, True),
        "bV": _prep_bias(bv, False),
        "cos": cos_t,
        "sin": sin_t,
    }

    xall = np.asarray(x, dtype=np.float32).reshape(N_CORES, TOK, D)
    in_maps = []
    for c in range(N_CORES):
        xT = np.ascontiguousarray(xall[c].T).astype(NPBF16)   # (D, TOK)
        in_maps.append({"xT": xT, **shared})
    return in_maps


def _assemble(results):
    outs = []
    for name in ("OK", "OQ", "OV"):
        full = np.concatenate(
            [np.asarray(results[c][name], dtype=np.float32).reshape(TOK, D)
             for c in range(N_CORES)], axis=0)
        outs.append(full.reshape(B, S, D))
    # reference returns (K, Q, V)
    return tuple(outs)


def _run(inputs, **run_kwargs):
    nc = _get_compiled()
    in_maps = _prep_inputs(**{k: np.asarray(v) for k, v in inputs.items()})
    last_err = None
    for _attempt in range(3):
        try:
            res = run_bass_kernel_spmd(nc, in_maps,
                                       core_ids=list(range(N_CORES)),
                                       **run_kwargs)
            return _assemble(res.results), res
        except Exception as e:  # transient NRT device errors — retry
            last_err = e
            import time
            time.sleep(2.0)
    raise last_err


def kernel(**inputs):
    outputs, _ = _run(inputs)
    return outputs

